# revision 22
# baseline (speedup 1.0000x reference)
"""GQA prefill attention (B=2, S=2048, D=2048, H=32, KV=8, HD=64) on 8 trn2 cores.

Sharding: tensor-parallel over heads. Core c owns q-heads [4c, 4c+4) and
kv-head c (n_rep=4), computes its partial of out = attn_out @ wo; host sums
the 8 partials (fp16 partials, fp64 accumulation).

Device kernel (per core, bf16 matmuls / fp32 PSUM):
  phase 1: QT[dh,s] = wq_c^T-chunks @ xT; KT/VT packed in one stream;
    VT transposed back to V[k,dh] on PE; RoPE via pair-swap permutation
    matmul + elementwise cos/sin tables.
  phase 2 (per (b, q-stripe), heads in 2 passes of 2):
    ST[k,q] = KT-chunk^T @ QT        (scores transposed, KT weights shared
                                      across the pass's 2 heads)
    P = exp(ST/8) (* mask tile)      (one ACT op per (kt, head-pair))
    OT[dh|1, q] += [V | 1]^T @ P     (accumulated over kt in PSUM; row 64
                                      is the softmax denominator)
    normalize: recip of row 64, PE rank-1 broadcast to all partitions,
    fused DVE multiply into the packed [2-head, q] wo-input layout
    out_partial[s,:] = packed-chunks^T @ wo_c  (fp16 output)
"""

import os
import sys

import numpy as np
import ml_dtypes

BF16 = ml_dtypes.bfloat16

B, S, D, H, KV, HD = 2, 2048, 2048, 32, 8, 64
NCORES = 8
HPC = H // NCORES  # 4 q-heads per core
QS_TILES = S // 512  # 4 q-stripes of 512 per batch
KT_TILES = S // 128  # 16 k-blocks of 128


def _host_prepare(x, wq, wk, wv, wo, freqs, mask):
    """Build per-core device inputs + the mask block schedule.

    All tensors are pre-tiled on the host into the exact [partition, ...]
    layouts the kernel DMAs, so every transfer is contiguous per partition
    (large descriptors instead of 512B-1KB scatter reads).
    """
    # xTt[b, st, p, c, s]: element = x[b, st*512+s, c*128+p]
    xTt = np.ascontiguousarray(
        x.transpose(0, 2, 1).reshape(B, 16, 128, S // 512, 512)
        .transpose(0, 3, 2, 1, 4)).astype(BF16)

    # RoPE tables in the [dh-on-partitions, s] layout used by QT/KT.
    # Two 64-row head copies stacked (head pairs live on 128 partitions).
    # rope: out[2j]   = t[2j] cos - t[2j+1] sin
    #       out[2j+1] = t[2j] sin + t[2j+1] cos
    # with swap(t)[d] = t[d^1]:  out[d] = t[d]*cos[d] + swap(t)[d]*sgn(d)*sin[d]
    c64 = np.cos(freqs.T).repeat(2, axis=0).astype(np.float64)  # [64, S]
    s64 = np.sin(freqs.T).repeat(2, axis=0).astype(np.float64)
    sgn = np.where(np.arange(HD) % 2 == 0, -1.0, 1.0)[:, None]
    cos_t = np.concatenate([c64, c64], axis=0).astype(BF16)           # [128, S]
    sin_t = np.concatenate([s64 * sgn, s64 * sgn], axis=0).astype(BF16)

    # Mask block schedule at [128 k x 512 q] granularity (same for all b, h).
    # Block (qs, kt): full (mask all zero), skip (all <= -30), or masked
    # (multiply exp'd P by exp(mask^T) tile).
    mt_tiles = []  # unique [128, 512] multiplier tiles
    mt_keys = {}
    sched = []  # per qs: list of (kt, mtile_idx | None, jlo)
    for qs in range(QS_TILES):
        lst = []
        for kt in range(KT_TILES):
            blk = mask[qs * 512:(qs + 1) * 512, kt * 128:(kt + 1) * 128]  # [q, k]
            if np.all(blk <= -30.0):
                continue
            # first 128-q subblock with any visible entry; only trust a
            # clean fully-masked prefix, else compute the whole stripe
            jmasked = [np.all(blk[j * 128:(j + 1) * 128] <= -30.0) for j in range(4)]
            jlo = 0
            while jlo < 4 and jmasked[jlo]:
                jlo += 1
            if any(jmasked[jlo:]):
                jlo = 0
            vis = blk[jlo * 128:]
            if np.all(vis == 0.0):
                lst.append((kt, None, jlo))
                continue
            tile_np = np.exp(blk.T.astype(np.float64)).astype(BF16)  # [128k, 512q]
            key = tile_np.tobytes()
            if key not in mt_keys:
                mt_keys[key] = len(mt_tiles)
                mt_tiles.append(tile_np)
            lst.append((kt, mt_keys[key], jlo))
        # if some q-subblock has no contributing kt at all, fall back to
        # full-width compute so its softmax denominator stays well-defined
        for j in range(4):
            if not any(e[2] <= j for e in lst):
                lst = [(kt, mi, 0) for (kt, mi, _) in lst]
                break
        # the OT-accumulation scheme needs the first block to cover the
        # full q-stripe (its start=True write initializes every column)
        assert lst and lst[0][2] == 0, "first visible kt must cover all q"
        assert all(a[2] <= b[2] for a, b in zip(lst, lst[1:])), "jlo monotone"
        sched.append(lst)
    if not mt_tiles:  # keep the input well-formed even if no masked blocks
        mt_tiles.append(np.ones((128, 512), dtype=BF16))
    mt = np.stack(mt_tiles)  # [U, 128, 512]

    mt_t = np.ascontiguousarray(mt.transpose(1, 0, 2))  # [128, U, 512]

    per_core = []
    for c in range(NCORES):
        wq_c = wq[:, c * HPC * HD:(c + 1) * HPC * HD]
        wkv_c = np.concatenate(
            [wk[:, c * HD:(c + 1) * HD], wv[:, c * HD:(c + 1) * HD]], axis=1)
        wo_c = wo[c * HPC * HD:(c + 1) * HPC * HD, :]
        per_core.append({
            "xT": xTt,
            # [p, c, m] tilings of the [d, m] weights (d = c*128 + p)
            "wq": np.ascontiguousarray(
                wq_c.reshape(16, 128, HPC * HD).transpose(1, 0, 2)).astype(BF16),
            "wkv": np.ascontiguousarray(
                wkv_c.reshape(16, 128, 2 * HD).transpose(1, 0, 2)).astype(BF16),
            # [p, g, n] tiling of wo (attn-dim = g*128 + p)
            "wo": np.ascontiguousarray(
                wo_c.reshape(2, 128, D).transpose(1, 0, 2)).astype(BF16),
            "cos": cos_t,
            "sin": sin_t,
            "mt": mt_t,
        })
    return per_core, sched, mt.shape[0]


def _untile_out(arr):
    """[B, 4, 4, 4, 128, 512] stripe tiles -> [B, S, D]."""
    return np.ascontiguousarray(
        arr.transpose(0, 1, 2, 4, 3, 5).reshape(B, S, D))


def _build_program(sched, U):
    import concourse.bass as bass
    import concourse.mybir as mybir
    import concourse.tile as tile
    from concourse import bacc

    dt = mybir.dt
    bf, f32, f16 = dt.bfloat16, dt.float32, dt.float16
    AF = mybir.ActivationFunctionType

    nc = bacc.Bacc("TRN2", target_bir_lowering=False, debug=False,
                   num_devices=NCORES)

    xT = nc.dram_tensor("xT", [B, S // 512, 128, DC_G := D // 128, 512], bf,
                        kind="ExternalInput")
    wq = nc.dram_tensor("wq", [128, D // 128, HPC * HD], bf, kind="ExternalInput")
    wkv = nc.dram_tensor("wkv", [128, D // 128, 2 * HD], bf, kind="ExternalInput")
    wo = nc.dram_tensor("wo", [128, 2, D], bf, kind="ExternalInput")
    cos = nc.dram_tensor("cos", [128, S], bf, kind="ExternalInput")
    sin = nc.dram_tensor("sin", [128, S], bf, kind="ExternalInput")
    mt = nc.dram_tensor("mt", [128, U, 512], bf, kind="ExternalInput")
    out = nc.dram_tensor("out", [B, QS_TILES, 4, 4, 128, 512], f16,
                         kind="ExternalOutput")

    # pair-swap permutation (block-diag over the two stacked 64-row heads)
    perm_np = np.zeros((128, 128), dtype=BF16)
    for d in range(128):
        perm_np[d ^ 1, d] = 1
    perm_dram = nc.inline_tensor(np.ascontiguousarray(perm_np), name="perm")
    ident_dram = nc.inline_tensor(np.eye(128, dtype=BF16), name="ident")

    DC = D // 128  # 16 contraction chunks

    with tile.TileContext(nc) as tc:
        with tc.tile_pool(name="const", bufs=1) as cp:
            wq_sb = cp.tile([128, DC, HPC * HD], bf)
            nc.sync.dma_start(wq_sb[:], wq.ap())
            wkv_sb = cp.tile([128, DC, 2 * HD], bf)
            nc.sync.dma_start(wkv_sb[:], wkv.ap())
            wo_sb = cp.tile([128, 2, D], bf)
            nc.sync.dma_start(wo_sb[:], wo.ap())
            cos_sb = cp.tile([128, S], bf)
            nc.sync.dma_start(cos_sb[:], cos.ap())
            sin_sb = cp.tile([128, S], bf)
            nc.sync.dma_start(sin_sb[:], sin.ap())
            mt_sb = cp.tile([128, U, 512], bf)
            nc.sync.dma_start(mt_sb[:], mt.ap())
            perm_sb = cp.tile([128, 128], bf)
            nc.sync.dma_start(perm_sb[:], perm_dram.ap())
            ident_sb = cp.tile([128, 128], bf)
            nc.sync.dma_start(ident_sb[:], ident_dram.ap())
            ones_sb = cp.tile([128, 128], f32)
            nc.vector.memset(ones_sb[:], 1.0)

            qt_sb = cp.tile([64, B, HPC, S], bf)   # [dh, b, head, s] (base-0)
            kt_sb = cp.tile([64, B, S], bf)        # [dh, b, s] (base-0)
            vone_sb = cp.tile([128, B, KT_TILES, HD + 1], bf)  # [k%128, b, kt, dh|1]
            nc.vector.memset(vone_sb[:, :, :, HD:HD + 1], 1.0)

            # ---------------- phase 1: projections + rope ----------------
            with (
                tc.tile_pool(name="xt", bufs=2) as xp,
                tc.tile_pool(name="raw", bufs=2) as rawp,
                tc.tile_pool(name="rtmp", bufs=2) as rtp,
                tc.tile_pool(name="ps_q0", bufs=2, space="PSUM") as pq0,
                tc.tile_pool(name="ps_q1", bufs=2, space="PSUM") as pq1,
                tc.tile_pool(name="ps_kv", bufs=2, space="PSUM") as pkv,
                tc.tile_pool(name="ps_sw", bufs=1, space="PSUM") as psw,
                tc.tile_pool(name="ps_vt", bufs=1, space="PSUM") as pvt,
            ):
                def ph1_tail(b, st, s0, q0r, q1r, kvr):
                    """V transpose + rope for one (b, st); deferred one
                    iteration so its PSUM-copy dependencies are long ready."""
                    for j in range(4):
                        vtp = pvt.tile([128, HD], bf)
                        nc.tensor.transpose(vtp[:], kvr[64:128, j * 128:(j + 1) * 128],
                                            ident_sb[64:128, 64:128])
                        nc.vector.tensor_copy(vone_sb[:, b, 4 * st + j, 0:HD], vtp[:])
                    # rope Q (both pairs)
                    for pb, qr in ((0, q0r), (1, q1r)):
                        swp = psw.tile([128, 512], f32, tag="sw")
                        nc.tensor.matmul(swp[:], lhsT=perm_sb[:], rhs=qr[:],
                                         start=True, stop=True)
                        t_sin = rtp.tile([128, 512], bf, tag="tsin")
                        nc.vector.tensor_mul(t_sin[:], swp[:], sin_sb[:, s0:s0 + 512])
                        t_cos = rtp.tile([128, 512], bf, tag="tcos")
                        nc.vector.tensor_mul(t_cos[:], qr[:], cos_sb[:, s0:s0 + 512])
                        nc.vector.tensor_add(qt_sb[:, b, 2 * pb, s0:s0 + 512],
                                             t_sin[0:64, :], t_cos[0:64, :])
                        nc.vector.tensor_add(qt_sb[:, b, 2 * pb + 1, s0:s0 + 512],
                                             t_sin[64:128, :], t_cos[64:128, :])
                    # rope K (rows 0:64 of kv)
                    ksw = psw.tile([64, 512], f32, tag="sw")
                    nc.tensor.matmul(ksw[:], lhsT=perm_sb[0:64, 0:64],
                                     rhs=kvr[0:64, :], start=True, stop=True)
                    k_sin = rtp.tile([64, 512], bf, tag="tsin")
                    nc.vector.tensor_mul(k_sin[:], ksw[:], sin_sb[0:64, s0:s0 + 512])
                    k_cos = rtp.tile([64, 512], bf, tag="tcos")
                    nc.vector.tensor_mul(k_cos[:], kvr[0:64, :], cos_sb[0:64, s0:s0 + 512])
                    nc.vector.tensor_add(kt_sb[:, b, s0:s0 + 512],
                                         k_sin[:], k_cos[:])

                tail_args = None
                for b in range(B):
                    for st in range(S // 512):
                        s0 = st * 512
                        xbig = xp.tile([128, DC, 512], bf)
                        nc.sync.dma_start(xbig[:], xT.ap()[b, st])
                        q0p = pq0.tile([128, 512], f32)
                        q1p = pq1.tile([128, 512], f32)
                        kvp = pkv.tile([128, 512], f32)
                        for dc in range(DC):
                            nc.tensor.matmul(q0p[:], lhsT=wq_sb[:, dc, 0:128],
                                             rhs=xbig[:, dc, :],
                                             start=(dc == 0), stop=(dc == DC - 1))
                            nc.tensor.matmul(q1p[:], lhsT=wq_sb[:, dc, 128:256],
                                             rhs=xbig[:, dc, :],
                                             start=(dc == 0), stop=(dc == DC - 1))
                            nc.tensor.matmul(kvp[:], lhsT=wkv_sb[:, dc, :],
                                             rhs=xbig[:, dc, :],
                                             start=(dc == 0), stop=(dc == DC - 1))
                        # raw copies to SBUF (also the swap-matmul inputs)
                        q0r = rawp.tile([128, 512], bf, tag="q0r")
                        nc.scalar.copy(q0r[:], q0p[:])
                        q1r = rawp.tile([128, 512], bf, tag="q1r")
                        nc.scalar.copy(q1r[:], q1p[:])
                        kvr = rawp.tile([128, 512], bf, tag="kvr")
                        nc.scalar.copy(kvr[:], kvp[:])
                        if tail_args is not None:
                            ph1_tail(*tail_args)
                        tail_args = (b, st, s0, q0r, q1r, kvr)
                ph1_tail(*tail_args)

            # ---------------- phase 2: attention + wo ----------------
            # PSUM budget (8 banks): scores pair-tiles [128,2,512] x2 bufs
            # (4 banks) + 2 OT accumulators (2 banks) + wo psum x2 (2 banks).
            with (
                tc.tile_pool(name="pp", bufs=3) as ppool,
                tc.tile_pool(name="pkd", bufs=2) as pkd,
                tc.tile_pool(name="rcp", bufs=2) as rcp,
                tc.tile_pool(name="scr", bufs=2) as scrp,
                tc.tile_pool(name="wsb", bufs=4) as wsp,
                tc.tile_pool(name="ps_s", bufs=2, space="PSUM") as pss,
                tc.tile_pool(name="ps_o0", bufs=1, space="PSUM") as po0,
                tc.tile_pool(name="ps_o1", bufs=1, space="PSUM") as po1,
                tc.tile_pool(name="ps_w", bufs=2, space="PSUM") as pwo,
            ):
                opool = (po0, po1)

                wo_units = []  # deferred wo sub-stages, drained inside kt loops

                def flush_wo(k):
                    for _ in range(min(k, len(wo_units))):
                        wo_units.pop(0)()

                def attn_pass(b, qs, pair):
                    """Scores+exp+AV for heads (2*pair, 2*pair+1); returns OT tiles."""
                    q0 = qs * 512
                    kts = sched[qs]
                    first_kt, last_kt = kts[0][0], kts[-1][0]
                    ots = [opool[i].tile([HD + 1, 512], f32, name=f"ot{i}")
                           for i in range(2)]
                    for kt, mi, jlo in kts:
                        c0 = jlo * 128
                        sp = pss.tile([128, 2, 512], f32, tag="sp")
                        for i in range(2):
                            h = 2 * pair + i
                            nc.tensor.matmul(
                                sp[:, i, c0:512],
                                lhsT=kt_sb[:, b, kt * 128:(kt + 1) * 128],
                                rhs=qt_sb[:, b, h, q0 + c0:q0 + 512],
                                start=True, stop=True)
                        pt = ppool.tile([128, 2, 512], bf, tag="pt")
                        nc.scalar.activation(pt[:, :, c0:512], sp[:, :, c0:512],
                                             AF.Exp, scale=1.0 / np.sqrt(HD))
                        if mi is not None:
                            # GPSIMD is otherwise idle; keep the mask mults off DVE
                            for i in range(2):
                                nc.gpsimd.tensor_mul(pt[:, i, c0:512], pt[:, i, c0:512],
                                                     mt_sb[:, mi, c0:512])
                        for i in range(2):
                            nc.tensor.matmul(
                                ots[i][:, c0:512],
                                lhsT=vone_sb[:, b, kt, :],
                                rhs=pt[:, i, c0:512],
                                start=(kt == first_kt), stop=(kt == last_kt))
                        # interleave prior-stripe wo work to keep the PE dense
                        # through the ACT-gated exp pipeline (HAM stays warm)
                        flush_wo(2)
                    return ots

                def norm_pack(pair, ots, packed):
                    """1/rowsum -> broadcast -> packed[dh-in-pair, pair, q]."""
                    bc = pss.tile([128, 2, 512], f32, tag="sp")  # borrow a slot
                    rc = rcp.tile([128, 2048], f32, tag="rc")
                    for i in range(2):
                        h = 2 * pair + i
                        pb, po = h // 2, (h % 2) * 64
                        # ~18-bit reciprocal, 5x faster than exact; denominators
                        # are finite and >= exp(max score) so edge cases can't hit.
                        # The staging copy also shifts the denominator row from
                        # partition 64 down to partition 0 so the custom DVE op
                        # and the K=1 broadcast matmul run on base-0 paths.
                        nc.vector.tensor_copy(rc[0:1, i * 512:(i + 1) * 512],
                                              ots[i][64:65, :])
                        nc.vector.reciprocal_approx_fast(
                            rc[0:1, 1024 + i * 512:1024 + (i + 1) * 512],
                            rc[0:1, i * 512:(i + 1) * 512])
                        # rank-1 broadcast of the recip row to all partitions
                        nc.tensor.matmul(bc[:, i, :], lhsT=ones_sb[0:1, :],
                                         rhs=rc[0:1, 1024 + i * 512:1024 + (i + 1) * 512],
                                         start=True, stop=True)
                        # DVE can read only one PSUM operand per op: stage OT in
                        # SBUF (the copy also partition-shifts the upper-half
                        # head into place), then multiply by the PSUM broadcast.
                        sc = scrp.tile([128, 512], bf, tag="sc")
                        nc.vector.tensor_copy(sc[po:po + 64, :], ots[i][0:64, :])
                        nc.vector.tensor_mul(packed[po:po + 64, pb, :],
                                             sc[po:po + 64, :], bc[po:po + 64, i, :])

                def make_wo_unit(b, qs, packed, j, nb):
                    def unit():
                        wp = pwo.tile([128, 512], f32)
                        nc.tensor.matmul(wp[:], lhsT=packed[:, 0, j * 128:(j + 1) * 128],
                                         rhs=wo_sb[:, 0, nb * 512:(nb + 1) * 512],
                                         start=True, stop=False)
                        nc.tensor.matmul(wp[:], lhsT=packed[:, 1, j * 128:(j + 1) * 128],
                                         rhs=wo_sb[:, 1, nb * 512:(nb + 1) * 512],
                                         start=False, stop=True)
                        wsb = wsp.tile([128, 512], f16)
                        nc.vector.tensor_copy(wsb[:], wp[:])
                        nc.sync.dma_start(out.ap()[b, qs, j, nb], wsb[:])
                    return unit

                for b in range(B):
                    for qs in range(QS_TILES):
                        packed = pkd.tile([128, 2, 512], bf, tag="packed")
                        ots = attn_pass(b, qs, 0)
                        norm_pack(0, ots, packed)
                        ots = attn_pass(b, qs, 1)
                        norm_pack(1, ots, packed)
                        wo_units.extend(make_wo_unit(b, qs, packed, j, nb)
                                        for j in range(4) for nb in range(4))
                while wo_units:
                    flush_wo(4)
    nc.compile()
    return nc


def kernel(x, wq, wk, wv, wo, freqs, mask, start_pos):
    sys.path.insert(0, "/opt/trn_rl_repo")
    from concourse.bass_utils import run_bass_kernel_spmd

    x = np.asarray(x, dtype=np.float32)
    per_core, sched, U = _host_prepare(
        x, np.asarray(wq, np.float32), np.asarray(wk, np.float32),
        np.asarray(wv, np.float32), np.asarray(wo, np.float32),
        np.asarray(freqs, np.float32), np.asarray(mask, np.float32))

    nc = _build_program(sched, U)

    trace = bool(int(os.environ.get("BASSKERNEL_TRACE", "0")))
    if trace and "antenv.axon_hooks" not in sys.modules:
        # profile-hook shim (the trimmed antenv package lacks axon_hooks)
        try:
            import types

            if "/root/.axon_site" not in sys.path:
                sys.path.insert(0, "/root/.axon_site")
            from trn_agent_boot.trn_boot import _ntff_profile_via_ctypes

            _hook = _ntff_profile_via_ctypes("/opt/axon/libaxon_pjrt.so")
            _mod = types.ModuleType("antenv.axon_hooks")
            _mod.get_axon_ntff_profile_hook = lambda: _hook
            _mod.set_axon_ntff_profile_hook = lambda h: None
            sys.modules["antenv.axon_hooks"] = _mod
        except Exception:
            trace = False
    res = run_bass_kernel_spmd(nc, per_core, core_ids=list(range(NCORES)),
                               trace=trace)
    if trace:
        kernel._last_exec_time_ns = res.exec_time_ns
        kernel._last_profile = res.profile_json
    acc = res.results[0]["out"].astype(np.float64)
    for c in range(1, NCORES):
        acc += res.results[c]["out"].astype(np.float64)
    return _untile_out(acc).astype(np.float32)


# revision 27
# speedup vs baseline: 1.0182x; 1.0182x over previous
"""GQA prefill attention (B=2, S=2048, D=2048, H=32, KV=8, HD=64) on 8 trn2 cores.

Sharding: tensor-parallel over heads. Core c owns q-heads [4c, 4c+4) and
kv-head c (n_rep=4), computes its partial of out = attn_out @ wo; host sums
the 8 partials (fp16 partials, fp64 accumulation).

Device kernel (per core, bf16 matmuls / fp32 PSUM):
  phase 1: QT[dh,s] = wq_c^T-chunks @ xT; KT/VT packed in one stream;
    VT transposed back to V[k,dh] on PE; RoPE via pair-swap permutation
    matmul + elementwise cos/sin tables.
  phase 2 (per (b, q-stripe), heads in 2 passes of 2):
    ST[k,q] = KT-chunk^T @ QT        (scores transposed, KT weights shared
                                      across the pass's 2 heads)
    P = exp(ST/8) (* mask tile)      (one ACT op per (kt, head-pair))
    OT[dh|1, q] += [V | 1]^T @ P     (accumulated over kt in PSUM; row 64
                                      is the softmax denominator)
    normalize: recip of row 64, PE rank-1 broadcast to all partitions,
    fused DVE multiply into the packed [2-head, q] wo-input layout
    out_partial[s,:] = packed-chunks^T @ wo_c  (fp16 output)
"""

import os
import sys

import numpy as np
import ml_dtypes

BF16 = ml_dtypes.bfloat16

B, S, D, H, KV, HD = 2, 2048, 2048, 32, 8, 64
NCORES = 8
HPC = H // NCORES  # 4 q-heads per core
QS_TILES = S // 512  # 4 q-stripes of 512 per batch
KT_TILES = S // 128  # 16 k-blocks of 128


def _host_prepare(x, wq, wk, wv, wo, freqs, mask):
    """Build per-core device inputs + the mask block schedule.

    All tensors are pre-tiled on the host into the exact [partition, ...]
    layouts the kernel DMAs, so every transfer is contiguous per partition
    (large descriptors instead of 512B-1KB scatter reads).
    """
    # xTt[b, st, p, c, s]: element = x[b, st*512+s, c*128+p]
    xTt = np.ascontiguousarray(
        x.transpose(0, 2, 1).reshape(B, 16, 128, S // 512, 512)
        .transpose(0, 3, 2, 1, 4)).astype(BF16)

    # RoPE tables in the [dh-on-partitions, s] layout used by QT/KT.
    # Two 64-row head copies stacked (head pairs live on 128 partitions).
    # rope: out[2j]   = t[2j] cos - t[2j+1] sin
    #       out[2j+1] = t[2j] sin + t[2j+1] cos
    # with swap(t)[d] = t[d^1]:  out[d] = t[d]*cos[d] + swap(t)[d]*sgn(d)*sin[d]
    c64 = np.cos(freqs.T).repeat(2, axis=0).astype(np.float64)  # [64, S]
    s64 = np.sin(freqs.T).repeat(2, axis=0).astype(np.float64)
    sgn = np.where(np.arange(HD) % 2 == 0, -1.0, 1.0)[:, None]
    cos_t = np.concatenate([c64, c64], axis=0).astype(BF16)           # [128, S]
    sin_t = np.concatenate([s64 * sgn, s64 * sgn], axis=0).astype(BF16)

    # Mask block schedule at [128 k x 512 q] granularity (same for all b, h).
    # Block (qs, kt): full (mask all zero), skip (all <= -30), or masked
    # (multiply exp'd P by exp(mask^T) tile).
    mt_tiles = []  # unique [128, 512] multiplier tiles
    mt_keys = {}
    sched = []  # per qs: list of (kt, mtile_idx | None, jlo)
    for qs in range(QS_TILES):
        lst = []
        for kt in range(KT_TILES):
            blk = mask[qs * 512:(qs + 1) * 512, kt * 128:(kt + 1) * 128]  # [q, k]
            if np.all(blk <= -30.0):
                continue
            # first 128-q subblock with any visible entry; only trust a
            # clean fully-masked prefix, else compute the whole stripe
            jmasked = [np.all(blk[j * 128:(j + 1) * 128] <= -30.0) for j in range(4)]
            jlo = 0
            while jlo < 4 and jmasked[jlo]:
                jlo += 1
            if any(jmasked[jlo:]):
                jlo = 0
            vis = blk[jlo * 128:]
            if np.all(vis == 0.0):
                lst.append((kt, None, jlo))
                continue
            tile_np = np.exp(blk.T.astype(np.float64)).astype(BF16)  # [128k, 512q]
            key = tile_np.tobytes()
            if key not in mt_keys:
                mt_keys[key] = len(mt_tiles)
                mt_tiles.append(tile_np)
            lst.append((kt, mt_keys[key], jlo))
        # if some q-subblock has no contributing kt at all, fall back to
        # full-width compute so its softmax denominator stays well-defined
        for j in range(4):
            if not any(e[2] <= j for e in lst):
                lst = [(kt, mi, 0) for (kt, mi, _) in lst]
                break
        # the OT-accumulation scheme needs the first block to cover the
        # full q-stripe (its start=True write initializes every column)
        assert lst and lst[0][2] == 0, "first visible kt must cover all q"
        assert all(a[2] <= b[2] for a, b in zip(lst, lst[1:])), "jlo monotone"
        sched.append(lst)
    if not mt_tiles:  # keep the input well-formed even if no masked blocks
        mt_tiles.append(np.ones((128, 512), dtype=BF16))
    mt = np.stack(mt_tiles)  # [U, 128, 512]

    mt_t = np.ascontiguousarray(mt.transpose(1, 0, 2))  # [128, U, 512]

    per_core = []
    for c in range(NCORES):
        wq_c = wq[:, c * HPC * HD:(c + 1) * HPC * HD]
        wkv_c = np.concatenate(
            [wk[:, c * HD:(c + 1) * HD], wv[:, c * HD:(c + 1) * HD]], axis=1)
        wo_c = wo[c * HPC * HD:(c + 1) * HPC * HD, :]
        per_core.append({
            "xT": xTt,
            # [p, c, m] tilings of the [d, m] weights (d = c*128 + p)
            "wq": np.ascontiguousarray(
                wq_c.reshape(16, 128, HPC * HD).transpose(1, 0, 2)).astype(BF16),
            "wkv": np.ascontiguousarray(
                wkv_c.reshape(16, 128, 2 * HD).transpose(1, 0, 2)).astype(BF16),
            # [p, g, n] tiling of wo (attn-dim = g*128 + p)
            "wo": np.ascontiguousarray(
                wo_c.reshape(2, 128, D).transpose(1, 0, 2)).astype(BF16),
            "cos": cos_t,
            "sin": sin_t,
            "mt": mt_t,
        })
    return per_core, sched, mt.shape[0]


def _untile_out(arr):
    """[B, 4, 4, 4, 128, 512] stripe tiles -> [B, S, D]."""
    return np.ascontiguousarray(
        arr.transpose(0, 1, 2, 4, 3, 5).reshape(B, S, D))


def _build_program(sched, U):
    import concourse.bass as bass
    import concourse.mybir as mybir
    import concourse.tile as tile
    from concourse import bacc

    dt = mybir.dt
    bf, f32, f16 = dt.bfloat16, dt.float32, dt.float16
    AF = mybir.ActivationFunctionType

    nc = bacc.Bacc("TRN2", target_bir_lowering=False, debug=False,
                   num_devices=NCORES)

    xT = nc.dram_tensor("xT", [B, S // 512, 128, DC_G := D // 128, 512], bf,
                        kind="ExternalInput")
    wq = nc.dram_tensor("wq", [128, D // 128, HPC * HD], bf, kind="ExternalInput")
    wkv = nc.dram_tensor("wkv", [128, D // 128, 2 * HD], bf, kind="ExternalInput")
    wo = nc.dram_tensor("wo", [128, 2, D], bf, kind="ExternalInput")
    cos = nc.dram_tensor("cos", [128, S], bf, kind="ExternalInput")
    sin = nc.dram_tensor("sin", [128, S], bf, kind="ExternalInput")
    mt = nc.dram_tensor("mt", [128, U, 512], bf, kind="ExternalInput")
    out = nc.dram_tensor("out", [B, QS_TILES, 4, 4, 128, 512], f16,
                         kind="ExternalOutput")

    # pair-swap permutation (block-diag over the two stacked 64-row heads)
    perm_np = np.zeros((128, 128), dtype=BF16)
    for d in range(128):
        perm_np[d ^ 1, d] = 1
    perm_dram = nc.inline_tensor(np.ascontiguousarray(perm_np), name="perm")
    ident_dram = nc.inline_tensor(np.eye(128, dtype=BF16), name="ident")

    DC = D // 128  # 16 contraction chunks

    with tile.TileContext(nc) as tc:
        with tc.tile_pool(name="const", bufs=1) as cp:
            wq_sb = cp.tile([128, DC, HPC * HD], bf)
            nc.sync.dma_start(wq_sb[:, 0:8, :], wq.ap()[:, 0:8])
            nc.sync.dma_start(wq_sb[:, 8:16, :], wq.ap()[:, 8:16])
            wkv_sb = cp.tile([128, DC, 2 * HD], bf)
            nc.sync.dma_start(wkv_sb[:], wkv.ap())
            wo_sb = cp.tile([128, 2, D], bf)
            nc.sync.dma_start(wo_sb[:], wo.ap())
            cos_sb = cp.tile([128, S], bf)
            nc.sync.dma_start(cos_sb[:], cos.ap())
            sin_sb = cp.tile([128, S], bf)
            nc.sync.dma_start(sin_sb[:], sin.ap())
            mt_sb = cp.tile([128, U, 512], bf)
            nc.sync.dma_start(mt_sb[:], mt.ap())
            perm_sb = cp.tile([128, 128], bf)
            nc.sync.dma_start(perm_sb[:], perm_dram.ap())
            ident_sb = cp.tile([128, 128], bf)
            nc.sync.dma_start(ident_sb[:], ident_dram.ap())
            ones_sb = cp.tile([128, 128], f32)
            nc.vector.memset(ones_sb[:], 1.0)

            qt_sb = cp.tile([64, B, HPC, S], bf)   # [dh, b, head, s] (base-0)
            kt_sb = cp.tile([64, B, S], bf)        # [dh, b, s] (base-0)
            vone_sb = cp.tile([128, B, KT_TILES, HD + 1], bf)  # [k%128, b, kt, dh|1]
            nc.vector.memset(vone_sb[:, :, :, HD:HD + 1], 1.0)

            # ---------------- phase 1: projections + rope ----------------
            SWAP_MASK = [i ^ 1 for i in range(32)]
            with (
                tc.tile_pool(name="xt", bufs=3) as xp,
                tc.tile_pool(name="raw", bufs=2) as rawp,
                tc.tile_pool(name="rtmp", bufs=2) as rtp,
                tc.tile_pool(name="ps_q0", bufs=2, space="PSUM") as pq0,
                tc.tile_pool(name="ps_q1", bufs=2, space="PSUM") as pq1,
                tc.tile_pool(name="ps_kv", bufs=2, space="PSUM") as pkv,
                tc.tile_pool(name="ps_vt", bufs=2, space="PSUM") as pvt,
            ):
                def ph1_tail(b, st, s0, q0r, q1r, kvr):
                    """V transpose + rope for one (b, st); deferred one
                    iteration so its PSUM-copy dependencies are long ready.
                    All 4 V transposes go into one PSUM tile (PE runs them
                    back-to-back) and drain with a single grouped copy; the
                    rope pair-swap runs on DVE (stream_shuffle), so the PE
                    side of the tail has no mid-tail DVE dependencies."""
                    vtp = pvt.tile([128, 4, HD], bf)
                    for j in range(4):
                        nc.tensor.transpose(vtp[:, j, :], kvr[64:128, j * 128:(j + 1) * 128],
                                            ident_sb[64:128, 64:128])
                    nc.vector.tensor_copy(vone_sb[:, b, 4 * st:4 * st + 4, 0:HD], vtp[:])
                    # rope Q (both pairs)
                    for pb, qr in ((0, q0r), (1, q1r)):
                        qsw = rtp.tile([128, 512], bf, tag="qsw")
                        nc.vector.stream_shuffle(qsw[:], qr[:], SWAP_MASK)
                        t_sin = rtp.tile([128, 512], bf, tag="tsin")
                        nc.vector.tensor_mul(t_sin[:], qsw[:], sin_sb[:, s0:s0 + 512])
                        t_cos = rtp.tile([128, 512], bf, tag="tcos")
                        nc.vector.tensor_mul(t_cos[:], qr[:], cos_sb[:, s0:s0 + 512])
                        nc.vector.tensor_add(qt_sb[:, b, 2 * pb, s0:s0 + 512],
                                             t_sin[0:64, :], t_cos[0:64, :])
                        nc.vector.tensor_add(qt_sb[:, b, 2 * pb + 1, s0:s0 + 512],
                                             t_sin[64:128, :], t_cos[64:128, :])
                    # rope K (rows 0:64 of kv)
                    ksw = rtp.tile([64, 512], bf, tag="ksw")
                    nc.vector.stream_shuffle(ksw[:], kvr[0:64, :], SWAP_MASK)
                    k_sin = rtp.tile([64, 512], bf, tag="tsin")
                    nc.vector.tensor_mul(k_sin[:], ksw[:], sin_sb[0:64, s0:s0 + 512])
                    k_cos = rtp.tile([64, 512], bf, tag="tcos")
                    nc.vector.tensor_mul(k_cos[:], kvr[0:64, :], cos_sb[0:64, s0:s0 + 512])
                    nc.vector.tensor_add(kt_sb[:, b, s0:s0 + 512],
                                         k_sin[:], k_cos[:])

                tail_args = None
                for b in range(B):
                    for st in range(S // 512):
                        s0 = st * 512
                        xbig = xp.tile([128, DC, 512], bf)
                        # split across two DMA queues for bandwidth
                        nc.sync.dma_start(xbig[:, 0:8, :], xT.ap()[b, st, :, 0:8])
                        nc.sync.dma_start(xbig[:, 8:16, :], xT.ap()[b, st, :, 8:16])
                        q0p = pq0.tile([128, 512], f32)
                        q1p = pq1.tile([128, 512], f32)
                        kvp = pkv.tile([128, 512], f32)
                        for dc in range(DC):
                            nc.tensor.matmul(q0p[:], lhsT=wq_sb[:, dc, 0:128],
                                             rhs=xbig[:, dc, :],
                                             start=(dc == 0), stop=(dc == DC - 1))
                            nc.tensor.matmul(q1p[:], lhsT=wq_sb[:, dc, 128:256],
                                             rhs=xbig[:, dc, :],
                                             start=(dc == 0), stop=(dc == DC - 1))
                            nc.tensor.matmul(kvp[:], lhsT=wkv_sb[:, dc, :],
                                             rhs=xbig[:, dc, :],
                                             start=(dc == 0), stop=(dc == DC - 1))
                        # raw copies to SBUF (also the swap-matmul inputs)
                        q0r = rawp.tile([128, 512], bf, tag="q0r")
                        nc.scalar.copy(q0r[:], q0p[:])
                        q1r = rawp.tile([128, 512], bf, tag="q1r")
                        nc.scalar.copy(q1r[:], q1p[:])
                        kvr = rawp.tile([128, 512], bf, tag="kvr")
                        nc.scalar.copy(kvr[:], kvp[:])
                        if tail_args is not None:
                            ph1_tail(*tail_args)
                        tail_args = (b, st, s0, q0r, q1r, kvr)
                ph1_tail(*tail_args)

            # ---------------- phase 2: attention + wo ----------------
            # PSUM budget (8 banks): scores pair-tiles [128,2,512] x2 bufs
            # (4 banks) + 2 OT accumulators (2 banks) + wo psum x2 (2 banks).
            with (
                tc.tile_pool(name="pp", bufs=3) as ppool,
                tc.tile_pool(name="pkd", bufs=2) as pkd,
                tc.tile_pool(name="rcp", bufs=2) as rcp,
                tc.tile_pool(name="scr", bufs=2) as scrp,
                tc.tile_pool(name="wsb", bufs=4) as wsp,
                tc.tile_pool(name="ps_s", bufs=2, space="PSUM") as pss,
                tc.tile_pool(name="ps_o0", bufs=1, space="PSUM") as po0,
                tc.tile_pool(name="ps_o1", bufs=1, space="PSUM") as po1,
                tc.tile_pool(name="ps_w", bufs=2, space="PSUM") as pwo,
            ):
                opool = (po0, po1)

                wo_units = []  # deferred wo sub-stages, drained inside kt loops

                def flush_wo(k):
                    for _ in range(min(k, len(wo_units))):
                        wo_units.pop(0)()

                def attn_pass(b, qs, pair):
                    """Scores+exp+AV for heads (2*pair, 2*pair+1); returns OT tiles."""
                    q0 = qs * 512
                    kts = sched[qs]
                    first_kt, last_kt = kts[0][0], kts[-1][0]
                    ots = [opool[i].tile([HD + 1, 512], f32, name=f"ot{i}")
                           for i in range(2)]

                    def emit_av(pt, c0, kt):
                        for i in range(2):
                            nc.tensor.matmul(
                                ots[i][:, c0:512],
                                lhsT=vone_sb[:, b, kt, :],
                                rhs=pt[:, i, c0:512],
                                start=(kt == first_kt), stop=(kt == last_kt))

                    # AV is software-pipelined one kt behind the scores so the
                    # in-order PE never sits through the exp(+mask) chain.
                    pending_av = None
                    for kt, mi, jlo in kts:
                        c0 = jlo * 128
                        sp = pss.tile([128, 2, 512], f32, tag="sp")
                        for i in range(2):
                            h = 2 * pair + i
                            nc.tensor.matmul(
                                sp[:, i, c0:512],
                                lhsT=kt_sb[:, b, kt * 128:(kt + 1) * 128],
                                rhs=qt_sb[:, b, h, q0 + c0:q0 + 512],
                                start=True, stop=True)
                        if pending_av is not None:
                            emit_av(*pending_av)
                        # interleave prior-stripe wo work to keep the PE dense
                        # through the ACT-gated exp pipeline (HAM stays warm)
                        flush_wo(2)
                        pt = ppool.tile([128, 2, 512], bf, tag="pt")
                        nc.scalar.activation(pt[:, :, c0:512], sp[:, :, c0:512],
                                             AF.Exp, scale=1.0 / np.sqrt(HD))
                        if mi is not None:
                            # GPSIMD is otherwise idle; keep the mask mults off DVE
                            for i in range(2):
                                nc.gpsimd.tensor_mul(pt[:, i, c0:512], pt[:, i, c0:512],
                                                     mt_sb[:, mi, c0:512])
                        pending_av = (pt, c0, kt)
                    emit_av(*pending_av)
                    return ots

                def norm_pack(pair, ots, packed):
                    """1/rowsum -> broadcast -> packed[dh-in-pair, pair, q]."""
                    flush_wo(3)  # PE filler while DVE runs the recip chain
                    bc = pss.tile([128, 2, 512], f32, tag="sp")  # borrow a slot
                    rc = rcp.tile([128, 2048], f32, tag="rc")
                    for i in range(2):
                        h = 2 * pair + i
                        pb, po = h // 2, (h % 2) * 64
                        # ~18-bit reciprocal, 5x faster than exact; denominators
                        # are finite and >= exp(max score) so edge cases can't hit.
                        # The staging copy also shifts the denominator row from
                        # partition 64 down to partition 0 so the custom DVE op
                        # and the K=1 broadcast matmul run on base-0 paths.
                        nc.vector.tensor_copy(rc[0:1, i * 512:(i + 1) * 512],
                                              ots[i][64:65, :])
                        nc.vector.reciprocal_approx_fast(
                            rc[0:1, 1024 + i * 512:1024 + (i + 1) * 512],
                            rc[0:1, i * 512:(i + 1) * 512])
                        # rank-1 broadcast of the recip row to all partitions
                        nc.tensor.matmul(bc[:, i, :], lhsT=ones_sb[0:1, :],
                                         rhs=rc[0:1, 1024 + i * 512:1024 + (i + 1) * 512],
                                         start=True, stop=True)
                        # DVE can read only one PSUM operand per op: stage OT in
                        # SBUF (the copy also partition-shifts the upper-half
                        # head into place), then multiply by the PSUM broadcast.
                        sc = scrp.tile([128, 512], bf, tag="sc")
                        nc.vector.tensor_copy(sc[po:po + 64, :], ots[i][0:64, :])
                        nc.vector.tensor_mul(packed[po:po + 64, pb, :],
                                             sc[po:po + 64, :], bc[po:po + 64, i, :])

                def make_wo_unit(b, qs, packed, j, nb):
                    def unit():
                        wp = pwo.tile([128, 512], f32)
                        nc.tensor.matmul(wp[:], lhsT=packed[:, 0, j * 128:(j + 1) * 128],
                                         rhs=wo_sb[:, 0, nb * 512:(nb + 1) * 512],
                                         start=True, stop=False)
                        nc.tensor.matmul(wp[:], lhsT=packed[:, 1, j * 128:(j + 1) * 128],
                                         rhs=wo_sb[:, 1, nb * 512:(nb + 1) * 512],
                                         start=False, stop=True)
                        wsb = wsp.tile([128, 512], f16)
                        nc.vector.tensor_copy(wsb[:], wp[:])
                        nc.sync.dma_start(out.ap()[b, qs, j, nb], wsb[:])
                    return unit

                for b in range(B):
                    for qs in range(QS_TILES):
                        packed = pkd.tile([128, 2, 512], bf, tag="packed")
                        ots = attn_pass(b, qs, 0)
                        norm_pack(0, ots, packed)
                        ots = attn_pass(b, qs, 1)
                        norm_pack(1, ots, packed)
                        wo_units.extend(make_wo_unit(b, qs, packed, j, nb)
                                        for j in range(4) for nb in range(4))
                while wo_units:
                    flush_wo(4)
    nc.compile()
    return nc


def kernel(x, wq, wk, wv, wo, freqs, mask, start_pos):
    sys.path.insert(0, "/opt/trn_rl_repo")
    from concourse.bass_utils import run_bass_kernel_spmd

    x = np.asarray(x, dtype=np.float32)
    per_core, sched, U = _host_prepare(
        x, np.asarray(wq, np.float32), np.asarray(wk, np.float32),
        np.asarray(wv, np.float32), np.asarray(wo, np.float32),
        np.asarray(freqs, np.float32), np.asarray(mask, np.float32))

    nc = _build_program(sched, U)

    trace = bool(int(os.environ.get("BASSKERNEL_TRACE", "0")))
    if trace and "antenv.axon_hooks" not in sys.modules:
        # profile-hook shim (the trimmed antenv package lacks axon_hooks)
        try:
            import types

            if "/root/.axon_site" not in sys.path:
                sys.path.insert(0, "/root/.axon_site")
            from trn_agent_boot.trn_boot import _ntff_profile_via_ctypes

            _hook = _ntff_profile_via_ctypes("/opt/axon/libaxon_pjrt.so")
            _mod = types.ModuleType("antenv.axon_hooks")
            _mod.get_axon_ntff_profile_hook = lambda: _hook
            _mod.set_axon_ntff_profile_hook = lambda h: None
            sys.modules["antenv.axon_hooks"] = _mod
        except Exception:
            trace = False
    res = run_bass_kernel_spmd(nc, per_core, core_ids=list(range(NCORES)),
                               trace=trace)
    if trace:
        kernel._last_exec_time_ns = res.exec_time_ns
        kernel._last_profile = res.profile_json
    acc = res.results[0]["out"].astype(np.float64)
    for c in range(1, NCORES):
        acc += res.results[c]["out"].astype(np.float64)
    return _untile_out(acc).astype(np.float32)


# revision 30
# speedup vs baseline: 1.1029x; 1.0831x over previous
"""GQA prefill attention (B=2, S=2048, D=2048, H=32, KV=8, HD=64) on 8 trn2 cores.

Sharding: tensor-parallel over heads. Core c owns q-heads [4c, 4c+4) and
kv-head c (n_rep=4), computes its partial of out = attn_out @ wo; host sums
the 8 partials (fp16 partials, fp64 accumulation).

Device kernel (per core, bf16 matmuls / fp32 PSUM):
  phase 1: QT[dh,s] = wq_c^T-chunks @ xT; KT/VT packed in one stream;
    VT transposed back to V[k,dh] on PE; RoPE via pair-swap permutation
    matmul + elementwise cos/sin tables.
  phase 2 (per (b, q-stripe), heads in 2 passes of 2):
    ST[k,q] = KT-chunk^T @ QT        (scores transposed, KT weights shared
                                      across the pass's 2 heads)
    P = exp(ST/8) (* mask tile)      (one ACT op per (kt, head-pair))
    OT[dh|1, q] += [V | 1]^T @ P     (accumulated over kt in PSUM; row 64
                                      is the softmax denominator)
    normalize: recip of row 64, PE rank-1 broadcast to all partitions,
    fused DVE multiply into the packed [2-head, q] wo-input layout
    out_partial[s,:] = packed-chunks^T @ wo_c  (fp16 output)
"""

import os
import sys

import numpy as np
import ml_dtypes

BF16 = ml_dtypes.bfloat16

B, S, D, H, KV, HD = 2, 2048, 2048, 32, 8, 64
NCORES = 8
HPC = H // NCORES  # 4 q-heads per core
QS_TILES = S // 512  # 4 q-stripes of 512 per batch
KT_TILES = S // 128  # 16 k-blocks of 128


def _host_prepare(x, wq, wk, wv, wo, freqs, mask):
    """Build per-core device inputs + the mask block schedule.

    All tensors are pre-tiled on the host into the exact [partition, ...]
    layouts the kernel DMAs, so every transfer is contiguous per partition
    (large descriptors instead of 512B-1KB scatter reads).
    """
    # xTt[b, st, p, c, s]: element = x[b, st*512+s, c*128+p]
    xTt = np.ascontiguousarray(
        x.transpose(0, 2, 1).reshape(B, 16, 128, S // 512, 512)
        .transpose(0, 3, 2, 1, 4)).astype(BF16)

    # RoPE tables in the [dh-on-partitions, s] layout used by QT/KT.
    # Two 64-row head copies stacked (head pairs live on 128 partitions).
    # rope: out[2j]   = t[2j] cos - t[2j+1] sin
    #       out[2j+1] = t[2j] sin + t[2j+1] cos
    # with swap(t)[d] = t[d^1]:  out[d] = t[d]*cos[d] + swap(t)[d]*sgn(d)*sin[d]
    c64 = np.cos(freqs.T).repeat(2, axis=0).astype(np.float64)  # [64, S]
    s64 = np.sin(freqs.T).repeat(2, axis=0).astype(np.float64)
    sgn = np.where(np.arange(HD) % 2 == 0, -1.0, 1.0)[:, None]
    cos_t = np.concatenate([c64, c64], axis=0).astype(BF16)           # [128, S]
    sin_t = np.concatenate([s64 * sgn, s64 * sgn], axis=0).astype(BF16)

    # Mask block schedule at [128 k x 512 q] granularity (same for all b, h).
    # Block (qs, kt): full (mask all zero), skip (all <= -30), or masked
    # (multiply exp'd P by exp(mask^T) tile).
    mt_tiles = []  # unique [128, 512] multiplier tiles
    mt_keys = {}
    sched = []  # per qs: list of (kt, mtile_idx | None, jlo)
    for qs in range(QS_TILES):
        lst = []
        for kt in range(KT_TILES):
            blk = mask[qs * 512:(qs + 1) * 512, kt * 128:(kt + 1) * 128]  # [q, k]
            if np.all(blk <= -30.0):
                continue
            # first 128-q subblock with any visible entry; only trust a
            # clean fully-masked prefix, else compute the whole stripe
            jmasked = [np.all(blk[j * 128:(j + 1) * 128] <= -30.0) for j in range(4)]
            jlo = 0
            while jlo < 4 and jmasked[jlo]:
                jlo += 1
            if any(jmasked[jlo:]):
                jlo = 0
            vis = blk[jlo * 128:]
            if np.all(vis == 0.0):
                lst.append((kt, None, jlo))
                continue
            tile_np = np.exp(blk.T.astype(np.float64)).astype(BF16)  # [128k, 512q]
            key = tile_np.tobytes()
            if key not in mt_keys:
                mt_keys[key] = len(mt_tiles)
                mt_tiles.append(tile_np)
            lst.append((kt, mt_keys[key], jlo))
        # if some q-subblock has no contributing kt at all, fall back to
        # full-width compute so its softmax denominator stays well-defined
        for j in range(4):
            if not any(e[2] <= j for e in lst):
                lst = [(kt, mi, 0) for (kt, mi, _) in lst]
                break
        # the OT-accumulation scheme needs the first block to cover the
        # full q-stripe (its start=True write initializes every column)
        assert lst and lst[0][2] == 0, "first visible kt must cover all q"
        assert all(a[2] <= b[2] for a, b in zip(lst, lst[1:])), "jlo monotone"
        sched.append(lst)
    if not mt_tiles:  # keep the input well-formed even if no masked blocks
        mt_tiles.append(np.ones((128, 512), dtype=BF16))
    mt = np.stack(mt_tiles)  # [U, 128, 512]

    mt_t = np.ascontiguousarray(mt.transpose(1, 0, 2))  # [128, U, 512]

    per_core = []
    for c in range(NCORES):
        wq_c = wq[:, c * HPC * HD:(c + 1) * HPC * HD]
        wkv_c = np.concatenate(
            [wk[:, c * HD:(c + 1) * HD], wv[:, c * HD:(c + 1) * HD]], axis=1)
        wo_c = wo[c * HPC * HD:(c + 1) * HPC * HD, :]
        per_core.append({
            "xT": xTt,
            # [p, c, m] tilings of the [d, m] weights (d = c*128 + p)
            "wq": np.ascontiguousarray(
                wq_c.reshape(16, 128, HPC * HD).transpose(1, 0, 2)).astype(BF16),
            "wkv": np.ascontiguousarray(
                wkv_c.reshape(16, 128, 2 * HD).transpose(1, 0, 2)).astype(BF16),
            # [p, g, n] tiling of wo (attn-dim = g*128 + p)
            "wo": np.ascontiguousarray(
                wo_c.reshape(2, 128, D).transpose(1, 0, 2)).astype(BF16),
            "cos": cos_t,
            "sin": sin_t,
            "mt": mt_t,
        })
    return per_core, sched, mt.shape[0]


def _untile_out(arr):
    """[B, 4, 4, 4, 128, 512] stripe tiles -> [B, S, D]."""
    return np.ascontiguousarray(
        arr.transpose(0, 1, 2, 4, 3, 5).reshape(B, S, D))


def _build_program(sched, U):
    import concourse.bass as bass
    import concourse.mybir as mybir
    import concourse.tile as tile
    from concourse import bacc

    dt = mybir.dt
    bf, f32, f16 = dt.bfloat16, dt.float32, dt.float16
    AF = mybir.ActivationFunctionType

    nc = bacc.Bacc("TRN2", target_bir_lowering=False, debug=False,
                   num_devices=NCORES)

    xT = nc.dram_tensor("xT", [B, S // 512, 128, DC_G := D // 128, 512], bf,
                        kind="ExternalInput")
    wq = nc.dram_tensor("wq", [128, D // 128, HPC * HD], bf, kind="ExternalInput")
    wkv = nc.dram_tensor("wkv", [128, D // 128, 2 * HD], bf, kind="ExternalInput")
    wo = nc.dram_tensor("wo", [128, 2, D], bf, kind="ExternalInput")
    cos = nc.dram_tensor("cos", [128, S], bf, kind="ExternalInput")
    sin = nc.dram_tensor("sin", [128, S], bf, kind="ExternalInput")
    mt = nc.dram_tensor("mt", [128, U, 512], bf, kind="ExternalInput")
    out = nc.dram_tensor("out", [B, QS_TILES, 4, 4, 128, 512], f16,
                         kind="ExternalOutput")

    # pair-swap permutation (block-diag over the two stacked 64-row heads)
    perm_np = np.zeros((128, 128), dtype=BF16)
    for d in range(128):
        perm_np[d ^ 1, d] = 1
    perm_dram = nc.inline_tensor(np.ascontiguousarray(perm_np), name="perm")
    ident_dram = nc.inline_tensor(np.eye(128, dtype=BF16), name="ident")

    DC = D // 128  # 16 contraction chunks

    with tile.TileContext(nc) as tc:
        with tc.tile_pool(name="const", bufs=1) as cp:
            wq_sb = cp.tile([128, DC, HPC * HD], bf)
            nc.sync.dma_start(wq_sb[:, 0:8, :], wq.ap()[:, 0:8])
            nc.sync.dma_start(wq_sb[:, 8:16, :], wq.ap()[:, 8:16])
            wkv_sb = cp.tile([128, DC, 2 * HD], bf)
            nc.sync.dma_start(wkv_sb[:], wkv.ap())
            wo_sb = cp.tile([128, 2, D], bf)
            nc.sync.dma_start(wo_sb[:], wo.ap())
            cos_sb = cp.tile([128, S], bf)
            nc.sync.dma_start(cos_sb[:], cos.ap())
            sin_sb = cp.tile([128, S], bf)
            nc.sync.dma_start(sin_sb[:], sin.ap())
            mt_sb = cp.tile([128, U, 512], bf)
            nc.sync.dma_start(mt_sb[:], mt.ap())
            perm_sb = cp.tile([128, 128], bf)
            nc.sync.dma_start(perm_sb[:], perm_dram.ap())
            ident_sb = cp.tile([128, 128], bf)
            nc.sync.dma_start(ident_sb[:], ident_dram.ap())
            ones_sb = cp.tile([128, 128], f32)
            nc.vector.memset(ones_sb[:], 1.0)

            # Q/K in row-packed layout: partitions 0:64 hold the pair's even
            # head, 64:128 the odd head (K is duplicated into both halves), so
            # the two scores matmuls of a pass run concurrently in different
            # PE row-strips (tile_position row packing).
            qt_sb = cp.tile([128, B, 2, S], bf)    # [dh|dh, b, pair, s]
            kt_sb = cp.tile([128, B, S], bf)       # [dh|dh(dup), b, s]
            vone_sb = cp.tile([128, B, KT_TILES, HD + 1], bf)  # [k%128, b, kt, dh|1]
            nc.vector.memset(vone_sb[:, :, :, HD:HD + 1], 1.0)

            # ---------------- phase 1: projections + rope ----------------
            SWAP_MASK = [i ^ 1 for i in range(32)]
            with (
                tc.tile_pool(name="xt", bufs=3) as xp,
                tc.tile_pool(name="raw", bufs=2) as rawp,
                tc.tile_pool(name="rtmp", bufs=2) as rtp,
                tc.tile_pool(name="ps_q0", bufs=2, space="PSUM") as pq0,
                tc.tile_pool(name="ps_q1", bufs=2, space="PSUM") as pq1,
                tc.tile_pool(name="ps_kv", bufs=2, space="PSUM") as pkv,
                tc.tile_pool(name="ps_vt", bufs=2, space="PSUM") as pvt,
            ):
                def ph1_tail(b, st, s0, q0r, q1r, kvr):
                    """V transpose + rope for one (b, st); deferred one
                    iteration so its PSUM-copy dependencies are long ready.
                    All 4 V transposes go into one PSUM tile (PE runs them
                    back-to-back) and drain with a single grouped copy; the
                    rope pair-swap runs on DVE (stream_shuffle), so the PE
                    side of the tail has no mid-tail DVE dependencies."""
                    vtp = pvt.tile([128, 4, HD], bf)
                    for j in range(4):
                        nc.tensor.transpose(vtp[:, j, :], kvr[64:128, j * 128:(j + 1) * 128],
                                            ident_sb[64:128, 64:128])
                    nc.vector.tensor_copy(vone_sb[:, b, 4 * st:4 * st + 4, 0:HD], vtp[:])
                    # rope Q (q0r/q1r already hold [even head | odd head] rows)
                    for pb, qr in ((0, q0r), (1, q1r)):
                        qsw = rtp.tile([128, 512], bf, tag="qsw")
                        nc.vector.stream_shuffle(qsw[:], qr[:], SWAP_MASK)
                        t_sin = rtp.tile([128, 512], bf, tag="tsin")
                        nc.vector.tensor_mul(t_sin[:], qsw[:], sin_sb[:, s0:s0 + 512])
                        t_cos = rtp.tile([128, 512], bf, tag="tcos")
                        nc.vector.tensor_mul(t_cos[:], qr[:], cos_sb[:, s0:s0 + 512])
                        nc.vector.tensor_add(qt_sb[:, b, pb, s0:s0 + 512],
                                             t_sin[:], t_cos[:])
                    # rope K (rows 0:64 of kv), then duplicate into rows 64:128
                    ksw = rtp.tile([64, 512], bf, tag="ksw")
                    nc.vector.stream_shuffle(ksw[:], kvr[0:64, :], SWAP_MASK)
                    k_sin = rtp.tile([64, 512], bf, tag="tsin")
                    nc.vector.tensor_mul(k_sin[:], ksw[:], sin_sb[0:64, s0:s0 + 512])
                    k_cos = rtp.tile([64, 512], bf, tag="tcos")
                    nc.vector.tensor_mul(k_cos[:], kvr[0:64, :], cos_sb[0:64, s0:s0 + 512])
                    nc.vector.tensor_add(kt_sb[0:64, b, s0:s0 + 512],
                                         k_sin[:], k_cos[:])
                    nc.vector.tensor_copy(kt_sb[64:128, b, s0:s0 + 512],
                                          kt_sb[0:64, b, s0:s0 + 512])

                tail_args = None
                for b in range(B):
                    for st in range(S // 512):
                        s0 = st * 512
                        xbig = xp.tile([128, DC, 512], bf)
                        # split across two DMA queues for bandwidth
                        nc.sync.dma_start(xbig[:, 0:8, :], xT.ap()[b, st, :, 0:8])
                        nc.sync.dma_start(xbig[:, 8:16, :], xT.ap()[b, st, :, 8:16])
                        q0p = pq0.tile([128, 512], f32)
                        q1p = pq1.tile([128, 512], f32)
                        kvp = pkv.tile([128, 512], f32)
                        for dc in range(DC):
                            nc.tensor.matmul(q0p[:], lhsT=wq_sb[:, dc, 0:128],
                                             rhs=xbig[:, dc, :],
                                             start=(dc == 0), stop=(dc == DC - 1))
                            nc.tensor.matmul(q1p[:], lhsT=wq_sb[:, dc, 128:256],
                                             rhs=xbig[:, dc, :],
                                             start=(dc == 0), stop=(dc == DC - 1))
                            nc.tensor.matmul(kvp[:], lhsT=wkv_sb[:, dc, :],
                                             rhs=xbig[:, dc, :],
                                             start=(dc == 0), stop=(dc == DC - 1))
                        # raw copies to SBUF (also the swap-matmul inputs)
                        q0r = rawp.tile([128, 512], bf, tag="q0r")
                        nc.scalar.copy(q0r[:], q0p[:])
                        q1r = rawp.tile([128, 512], bf, tag="q1r")
                        nc.scalar.copy(q1r[:], q1p[:])
                        kvr = rawp.tile([128, 512], bf, tag="kvr")
                        nc.scalar.copy(kvr[:], kvp[:])
                        if tail_args is not None:
                            ph1_tail(*tail_args)
                        tail_args = (b, st, s0, q0r, q1r, kvr)
                ph1_tail(*tail_args)

            # ---------------- phase 2: attention + wo ----------------
            # PSUM budget (8 banks): scores pair-tiles [128,2,512] x2 bufs
            # (4 banks) + 2 OT accumulators (2 banks) + wo psum x2 (2 banks).
            with (
                tc.tile_pool(name="pp", bufs=3) as ppool,
                tc.tile_pool(name="pkd", bufs=2) as pkd,
                tc.tile_pool(name="rcp", bufs=2) as rcp,
                tc.tile_pool(name="scr", bufs=2) as scrp,
                tc.tile_pool(name="wsb", bufs=4) as wsp,
                tc.tile_pool(name="ps_s", bufs=2, space="PSUM") as pss,
                tc.tile_pool(name="ps_o0", bufs=1, space="PSUM") as po0,
                tc.tile_pool(name="ps_o1", bufs=1, space="PSUM") as po1,
                tc.tile_pool(name="ps_w", bufs=2, space="PSUM") as pwo,
            ):
                opool = (po0, po1)

                wo_units = []  # deferred wo sub-stages, drained inside kt loops

                def flush_wo(k):
                    for _ in range(min(k, len(wo_units))):
                        wo_units.pop(0)()

                def attn_pass(b, qs, pair):
                    """Scores+exp+AV for heads (2*pair, 2*pair+1); returns OT tiles."""
                    q0 = qs * 512
                    kts = sched[qs]
                    first_kt, last_kt = kts[0][0], kts[-1][0]
                    ots = [opool[i].tile([HD + 1, 512], f32, name=f"ot{i}")
                           for i in range(2)]

                    def emit_av(pt, c0, kt):
                        for i in range(2):
                            nc.tensor.matmul(
                                ots[i][:, c0:512],
                                lhsT=vone_sb[:, b, kt, :],
                                rhs=pt[:, i, c0:512],
                                start=(kt == first_kt), stop=(kt == last_kt))

                    # AV is software-pipelined one kt behind the scores so the
                    # in-order PE never sits through the exp(+mask) chain.
                    pending_av = None
                    for kt, mi, jlo in kts:
                        c0 = jlo * 128
                        sp = pss.tile([128, 2, 512], f32, tag="sp")
                        for i in range(2):
                            r0 = i * 64  # row strip: the two heads' scores
                            nc.tensor.matmul(  # run concurrently on the PE
                                sp[:, i, c0:512],
                                lhsT=kt_sb[r0:r0 + 64, b, kt * 128:(kt + 1) * 128],
                                rhs=qt_sb[r0:r0 + 64, b, pair, q0 + c0:q0 + 512],
                                start=True, stop=True)
                        if pending_av is not None:
                            emit_av(*pending_av)
                        # interleave prior-stripe wo work to keep the PE dense
                        # through the ACT-gated exp pipeline (HAM stays warm)
                        flush_wo(2)
                        pt = ppool.tile([128, 2, 512], bf, tag="pt")
                        nc.scalar.activation(pt[:, :, c0:512], sp[:, :, c0:512],
                                             AF.Exp, scale=1.0 / np.sqrt(HD))
                        if mi is not None:
                            # GPSIMD is otherwise idle; keep the mask mults off DVE
                            for i in range(2):
                                nc.gpsimd.tensor_mul(pt[:, i, c0:512], pt[:, i, c0:512],
                                                     mt_sb[:, mi, c0:512])
                        pending_av = (pt, c0, kt)
                    emit_av(*pending_av)
                    return ots

                def norm_pack(pair, ots, packed):
                    """1/rowsum -> broadcast -> packed[dh-in-pair, pair, q]."""
                    flush_wo(3)  # PE filler while DVE runs the recip chain
                    bc = pss.tile([128, 2, 512], f32, tag="sp")  # borrow a slot
                    rc = rcp.tile([128, 2048], f32, tag="rc")
                    for i in range(2):
                        h = 2 * pair + i
                        pb, po = h // 2, (h % 2) * 64
                        # ~18-bit reciprocal, 5x faster than exact; denominators
                        # are finite and >= exp(max score) so edge cases can't hit.
                        # The staging copy also shifts the denominator row from
                        # partition 64 down to partition 0 so the custom DVE op
                        # and the K=1 broadcast matmul run on base-0 paths.
                        nc.vector.tensor_copy(rc[0:1, i * 512:(i + 1) * 512],
                                              ots[i][64:65, :])
                        nc.vector.reciprocal_approx_fast(
                            rc[0:1, 1024 + i * 512:1024 + (i + 1) * 512],
                            rc[0:1, i * 512:(i + 1) * 512])
                        # rank-1 broadcast of the recip row to all partitions
                        nc.tensor.matmul(bc[:, i, :], lhsT=ones_sb[0:1, :],
                                         rhs=rc[0:1, 1024 + i * 512:1024 + (i + 1) * 512],
                                         start=True, stop=True)
                        # DVE can read only one PSUM operand per op: stage OT in
                        # SBUF (the copy also partition-shifts the upper-half
                        # head into place), then multiply by the PSUM broadcast.
                        sc = scrp.tile([128, 512], bf, tag="sc")
                        nc.vector.tensor_copy(sc[po:po + 64, :], ots[i][0:64, :])
                        nc.vector.tensor_mul(packed[po:po + 64, pb, :],
                                             sc[po:po + 64, :], bc[po:po + 64, i, :])

                def make_wo_unit(b, qs, packed, j, nb):
                    def unit():
                        wp = pwo.tile([128, 512], f32)
                        nc.tensor.matmul(wp[:], lhsT=packed[:, 0, j * 128:(j + 1) * 128],
                                         rhs=wo_sb[:, 0, nb * 512:(nb + 1) * 512],
                                         start=True, stop=False)
                        nc.tensor.matmul(wp[:], lhsT=packed[:, 1, j * 128:(j + 1) * 128],
                                         rhs=wo_sb[:, 1, nb * 512:(nb + 1) * 512],
                                         start=False, stop=True)
                        wsb = wsp.tile([128, 512], f16)
                        nc.vector.tensor_copy(wsb[:], wp[:])
                        nc.sync.dma_start(out.ap()[b, qs, j, nb], wsb[:])
                    return unit

                for b in range(B):
                    for qs in range(QS_TILES):
                        packed = pkd.tile([128, 2, 512], bf, tag="packed")
                        ots = attn_pass(b, qs, 0)
                        norm_pack(0, ots, packed)
                        ots = attn_pass(b, qs, 1)
                        norm_pack(1, ots, packed)
                        wo_units.extend(make_wo_unit(b, qs, packed, j, nb)
                                        for j in range(4) for nb in range(4))
                while wo_units:
                    flush_wo(4)
    nc.compile()
    return nc


def kernel(x, wq, wk, wv, wo, freqs, mask, start_pos):
    sys.path.insert(0, "/opt/trn_rl_repo")
    from concourse.bass_utils import run_bass_kernel_spmd

    x = np.asarray(x, dtype=np.float32)
    per_core, sched, U = _host_prepare(
        x, np.asarray(wq, np.float32), np.asarray(wk, np.float32),
        np.asarray(wv, np.float32), np.asarray(wo, np.float32),
        np.asarray(freqs, np.float32), np.asarray(mask, np.float32))

    nc = _build_program(sched, U)

    trace = bool(int(os.environ.get("BASSKERNEL_TRACE", "0")))
    if trace and "antenv.axon_hooks" not in sys.modules:
        # profile-hook shim (the trimmed antenv package lacks axon_hooks)
        try:
            import types

            if "/root/.axon_site" not in sys.path:
                sys.path.insert(0, "/root/.axon_site")
            from trn_agent_boot.trn_boot import _ntff_profile_via_ctypes

            _hook = _ntff_profile_via_ctypes("/opt/axon/libaxon_pjrt.so")
            _mod = types.ModuleType("antenv.axon_hooks")
            _mod.get_axon_ntff_profile_hook = lambda: _hook
            _mod.set_axon_ntff_profile_hook = lambda h: None
            sys.modules["antenv.axon_hooks"] = _mod
        except Exception:
            trace = False
    res = run_bass_kernel_spmd(nc, per_core, core_ids=list(range(NCORES)),
                               trace=trace)
    if trace:
        kernel._last_exec_time_ns = res.exec_time_ns
        kernel._last_profile = res.profile_json
    acc = res.results[0]["out"].astype(np.float64)
    for c in range(1, NCORES):
        acc += res.results[c]["out"].astype(np.float64)
    return _untile_out(acc).astype(np.float32)


# revision 35
# speedup vs baseline: 1.1530x; 1.0454x over previous
"""GQA prefill attention (B=2, S=2048, D=2048, H=32, KV=8, HD=64) on 8 trn2 cores.

Sharding: tensor-parallel over heads. Core c owns q-heads [4c, 4c+4) and
kv-head c (n_rep=4), computes its partial of out = attn_out @ wo; host sums
the 8 partials (fp16 partials, fp64 accumulation).

Device kernel (per core, bf16 matmuls / fp32 PSUM):
  phase 1: QT[dh,s] = wq_c^T-chunks @ xT; KT/VT packed in one stream;
    VT transposed back to V[k,dh] on PE; RoPE via pair-swap permutation
    matmul + elementwise cos/sin tables.
  phase 2 (per (b, q-stripe), heads in 2 passes of 2):
    ST[k,q] = KT-chunk^T @ QT        (scores transposed, KT weights shared
                                      across the pass's 2 heads)
    P = exp(ST/8) (* mask tile)      (one ACT op per (kt, head-pair))
    OT[dh|1, q] += [V | 1]^T @ P     (accumulated over kt in PSUM; row 64
                                      is the softmax denominator)
    normalize: recip of row 64, PE rank-1 broadcast to all partitions,
    fused DVE multiply into the packed [2-head, q] wo-input layout
    out_partial[s,:] = packed-chunks^T @ wo_c  (fp16 output)
"""

import os
import sys

import numpy as np
import ml_dtypes

BF16 = ml_dtypes.bfloat16

B, S, D, H, KV, HD = 2, 2048, 2048, 32, 8, 64
NCORES = 8
HPC = H // NCORES  # 4 q-heads per core
QS_TILES = S // 512  # 4 q-stripes of 512 per batch
KT_TILES = S // 128  # 16 k-blocks of 128


def _host_prepare(x, wq, wk, wv, wo, freqs, mask):
    """Build per-core device inputs + the mask block schedule.

    All tensors are pre-tiled on the host into the exact [partition, ...]
    layouts the kernel DMAs, so every transfer is contiguous per partition
    (large descriptors instead of 512B-1KB scatter reads).
    """
    # xTt[b, st, p, c, s]: element = x[b, st*512+s, c*128+p]
    xTt = np.ascontiguousarray(
        x.transpose(0, 2, 1).reshape(B, 16, 128, S // 512, 512)
        .transpose(0, 3, 2, 1, 4)).astype(BF16)

    # RoPE tables in the [dh-on-partitions, s] layout used by QT/KT.
    # Two 64-row head copies stacked (head pairs live on 128 partitions).
    # rope: out[2j]   = t[2j] cos - t[2j+1] sin
    #       out[2j+1] = t[2j] sin + t[2j+1] cos
    # with swap(t)[d] = t[d^1]:  out[d] = t[d]*cos[d] + swap(t)[d]*sgn(d)*sin[d]
    c64 = np.cos(freqs.T).repeat(2, axis=0).astype(np.float64)  # [64, S]
    s64 = np.sin(freqs.T).repeat(2, axis=0).astype(np.float64)
    sgn = np.where(np.arange(HD) % 2 == 0, -1.0, 1.0)[:, None]
    cos_t = np.concatenate([c64, c64], axis=0).astype(BF16)           # [128, S]
    sin_t = np.concatenate([s64 * sgn, s64 * sgn], axis=0).astype(BF16)

    # Mask block schedule at [128 k x 512 q] granularity (same for all b, h).
    # Block (qs, kt): full (mask all zero), skip (all <= -30), or masked
    # (multiply exp'd P by exp(mask^T) tile).
    mt_tiles = []  # unique [128, 512] multiplier tiles
    mt_keys = {}
    sched = []  # per qs: list of (kt, mtile_idx | None, jlo)
    for qs in range(QS_TILES):
        lst = []
        for kt in range(KT_TILES):
            blk = mask[qs * 512:(qs + 1) * 512, kt * 128:(kt + 1) * 128]  # [q, k]
            if np.all(blk <= -30.0):
                continue
            # first 128-q subblock with any visible entry; only trust a
            # clean fully-masked prefix, else compute the whole stripe
            jmasked = [np.all(blk[j * 128:(j + 1) * 128] <= -30.0) for j in range(4)]
            jlo = 0
            while jlo < 4 and jmasked[jlo]:
                jlo += 1
            if any(jmasked[jlo:]):
                jlo = 0
            vis = blk[jlo * 128:]
            if np.all(vis == 0.0):
                lst.append((kt, None, jlo))
                continue
            tile_np = np.exp(blk.T.astype(np.float64)).astype(BF16)  # [128k, 512q]
            key = tile_np.tobytes()
            if key not in mt_keys:
                mt_keys[key] = len(mt_tiles)
                mt_tiles.append(tile_np)
            lst.append((kt, mt_keys[key], jlo))
        # if some q-subblock has no contributing kt at all, fall back to
        # full-width compute so its softmax denominator stays well-defined
        for j in range(4):
            if not any(e[2] <= j for e in lst):
                lst = [(kt, mi, 0) for (kt, mi, _) in lst]
                break
        # the OT-accumulation scheme needs the first block to cover the
        # full q-stripe (its start=True write initializes every column)
        assert lst and lst[0][2] == 0, "first visible kt must cover all q"
        assert all(a[2] <= b[2] for a, b in zip(lst, lst[1:])), "jlo monotone"
        sched.append(lst)
    if not mt_tiles:  # keep the input well-formed even if no masked blocks
        mt_tiles.append(np.ones((128, 512), dtype=BF16))
    mt = np.stack(mt_tiles)  # [U, 128, 512]

    mt_t = np.ascontiguousarray(mt.transpose(1, 0, 2))  # [128, U, 512]

    per_core = []
    for c in range(NCORES):
        wq_c = wq[:, c * HPC * HD:(c + 1) * HPC * HD]
        wkv_c = np.concatenate(
            [wk[:, c * HD:(c + 1) * HD], wv[:, c * HD:(c + 1) * HD]], axis=1)
        wo_c = wo[c * HPC * HD:(c + 1) * HPC * HD, :]
        per_core.append({
            "xT": xTt,
            # [p, c, m] tilings of the [d, m] weights (d = c*128 + p)
            "wq": np.ascontiguousarray(
                wq_c.reshape(16, 128, HPC * HD).transpose(1, 0, 2)).astype(BF16),
            "wkv": np.ascontiguousarray(
                wkv_c.reshape(16, 128, 2 * HD).transpose(1, 0, 2)).astype(BF16),
            # [p, g, n] tiling of wo (attn-dim = g*128 + p)
            "wo": np.ascontiguousarray(
                wo_c.reshape(2, 128, D).transpose(1, 0, 2)).astype(BF16),
            "cos": cos_t,
            "sin": sin_t,
            "mt": mt_t,
        })
    return per_core, sched, mt.shape[0]


def _untile_out(arr):
    """[B, 4, 4, 4, 128, 512] stripe tiles -> [B, S, D]."""
    return np.ascontiguousarray(
        arr.transpose(0, 1, 2, 4, 3, 5).reshape(B, S, D))


def _build_program(sched, U):
    import concourse.bass as bass
    import concourse.mybir as mybir
    import concourse.tile as tile
    from concourse import bacc

    dt = mybir.dt
    bf, f32, f16 = dt.bfloat16, dt.float32, dt.float16
    AF = mybir.ActivationFunctionType

    nc = bacc.Bacc("TRN2", target_bir_lowering=False, debug=False,
                   num_devices=NCORES)

    xT = nc.dram_tensor("xT", [B, S // 512, 128, DC_G := D // 128, 512], bf,
                        kind="ExternalInput")
    wq = nc.dram_tensor("wq", [128, D // 128, HPC * HD], bf, kind="ExternalInput")
    wkv = nc.dram_tensor("wkv", [128, D // 128, 2 * HD], bf, kind="ExternalInput")
    wo = nc.dram_tensor("wo", [128, 2, D], bf, kind="ExternalInput")
    cos = nc.dram_tensor("cos", [128, S], bf, kind="ExternalInput")
    sin = nc.dram_tensor("sin", [128, S], bf, kind="ExternalInput")
    mt = nc.dram_tensor("mt", [128, U, 512], bf, kind="ExternalInput")
    out = nc.dram_tensor("out", [B, QS_TILES, 4, 4, 128, 512], f16,
                         kind="ExternalOutput")

    # pair-swap permutation (block-diag over the two stacked 64-row heads)
    perm_np = np.zeros((128, 128), dtype=BF16)
    for d in range(128):
        perm_np[d ^ 1, d] = 1
    perm_dram = nc.inline_tensor(np.ascontiguousarray(perm_np), name="perm")
    ident_dram = nc.inline_tensor(np.eye(128, dtype=BF16), name="ident")

    DC = D // 128  # 16 contraction chunks

    with tile.TileContext(nc) as tc:
        with tc.tile_pool(name="const", bufs=1) as cp:
            wq_sb = cp.tile([128, DC, HPC * HD], bf)
            nc.sync.dma_start(wq_sb[:, 0:8, :], wq.ap()[:, 0:8])
            nc.sync.dma_start(wq_sb[:, 8:16, :], wq.ap()[:, 8:16])
            wkv_sb = cp.tile([128, DC, 2 * HD], bf)
            nc.sync.dma_start(wkv_sb[:], wkv.ap())
            wo_sb = cp.tile([128, 2, D], bf)
            nc.sync.dma_start(wo_sb[:], wo.ap())
            cos_sb = cp.tile([128, S], bf)
            nc.sync.dma_start(cos_sb[:], cos.ap())
            sin_sb = cp.tile([128, S], bf)
            nc.sync.dma_start(sin_sb[:], sin.ap())
            mt_sb = cp.tile([128, U, 512], bf)
            nc.sync.dma_start(mt_sb[:], mt.ap())
            perm_sb = cp.tile([128, 128], bf)
            nc.sync.dma_start(perm_sb[:], perm_dram.ap())
            ident_sb = cp.tile([128, 128], bf)
            nc.sync.dma_start(ident_sb[:], ident_dram.ap())
            ones_sb = cp.tile([128, 128], f32)
            nc.vector.memset(ones_sb[:], 1.0)

            # Q/K in row-packed layout: partitions 0:64 hold the pair's even
            # head, 64:128 the odd head (K is duplicated into both halves), so
            # the two scores matmuls of a pass run concurrently in different
            # PE row-strips (tile_position row packing).
            qt_sb = cp.tile([128, B, 2, S], bf)    # [dh|dh, b, pair, s]
            kt_sb = cp.tile([128, B, S], bf)       # [dh|dh(dup), b, s]
            vone_sb = cp.tile([128, B, KT_TILES, HD + 1], bf)  # [k%128, b, kt, dh|1]
            nc.vector.memset(vone_sb[:, :, :, HD:HD + 1], 1.0)

            # ---------------- phase 1: projections + rope ----------------
            SWAP_MASK = [i ^ 1 for i in range(32)]
            with (
                tc.tile_pool(name="xt", bufs=3) as xp,
                tc.tile_pool(name="raw", bufs=2) as rawp,
                tc.tile_pool(name="rtmp", bufs=2) as rtp,
                tc.tile_pool(name="ps_q0", bufs=2, space="PSUM") as pq0,
                tc.tile_pool(name="ps_q1", bufs=2, space="PSUM") as pq1,
                tc.tile_pool(name="ps_kv", bufs=2, space="PSUM") as pkv,
                tc.tile_pool(name="ps_vt", bufs=2, space="PSUM") as pvt,
            ):
                def ph1_tail(b, st, s0, q0r, q1r, kvr):
                    """V transpose + rope for one (b, st); deferred one
                    iteration so its PSUM-copy dependencies are long ready.
                    All 4 V transposes go into one PSUM tile (PE runs them
                    back-to-back) and drain with a single grouped copy; the
                    rope pair-swap runs on DVE (stream_shuffle), so the PE
                    side of the tail has no mid-tail DVE dependencies."""
                    vtp = pvt.tile([128, 4, HD], bf)
                    for j in range(4):
                        nc.tensor.transpose(vtp[:, j, :], kvr[64:128, j * 128:(j + 1) * 128],
                                            ident_sb[64:128, 64:128])
                    nc.vector.tensor_copy(vone_sb[:, b, 4 * st:4 * st + 4, 0:HD], vtp[:])
                    # rope Q (q0r/q1r already hold [even head | odd head] rows)
                    for pb, qr in ((0, q0r), (1, q1r)):
                        qsw = rtp.tile([128, 512], bf, tag="qsw")
                        nc.vector.stream_shuffle(qsw[:], qr[:], SWAP_MASK)
                        t_sin = rtp.tile([128, 512], bf, tag="tsin")
                        nc.vector.tensor_mul(t_sin[:], qsw[:], sin_sb[:, s0:s0 + 512])
                        t_cos = rtp.tile([128, 512], bf, tag="tcos")
                        nc.vector.tensor_mul(t_cos[:], qr[:], cos_sb[:, s0:s0 + 512])
                        nc.vector.tensor_add(qt_sb[:, b, pb, s0:s0 + 512],
                                             t_sin[:], t_cos[:])
                    # rope K (rows 0:64 of kv), then duplicate into rows 64:128
                    ksw = rtp.tile([64, 512], bf, tag="ksw")
                    nc.vector.stream_shuffle(ksw[:], kvr[0:64, :], SWAP_MASK)
                    k_sin = rtp.tile([64, 512], bf, tag="tsin")
                    nc.vector.tensor_mul(k_sin[:], ksw[:], sin_sb[0:64, s0:s0 + 512])
                    k_cos = rtp.tile([64, 512], bf, tag="tcos")
                    nc.vector.tensor_mul(k_cos[:], kvr[0:64, :], cos_sb[0:64, s0:s0 + 512])
                    nc.vector.tensor_add(kt_sb[0:64, b, s0:s0 + 512],
                                         k_sin[:], k_cos[:])
                    nc.vector.tensor_copy(kt_sb[64:128, b, s0:s0 + 512],
                                          kt_sb[0:64, b, s0:s0 + 512])

                tail_args = None
                for b in range(B):
                    for st in range(S // 512):
                        s0 = st * 512
                        xbig = xp.tile([128, DC, 512], bf)
                        # split across two DMA queues for bandwidth
                        nc.sync.dma_start(xbig[:, 0:8, :], xT.ap()[b, st, :, 0:8])
                        nc.sync.dma_start(xbig[:, 8:16, :], xT.ap()[b, st, :, 8:16])
                        q0p = pq0.tile([128, 512], f32)
                        q1p = pq1.tile([128, 512], f32)
                        kvp = pkv.tile([128, 512], f32)
                        for dc in range(DC):
                            nc.tensor.matmul(q0p[:], lhsT=wq_sb[:, dc, 0:128],
                                             rhs=xbig[:, dc, :],
                                             start=(dc == 0), stop=(dc == DC - 1))
                            nc.tensor.matmul(q1p[:], lhsT=wq_sb[:, dc, 128:256],
                                             rhs=xbig[:, dc, :],
                                             start=(dc == 0), stop=(dc == DC - 1))
                            nc.tensor.matmul(kvp[:], lhsT=wkv_sb[:, dc, :],
                                             rhs=xbig[:, dc, :],
                                             start=(dc == 0), stop=(dc == DC - 1))
                        # raw copies to SBUF (also the swap-matmul inputs)
                        q0r = rawp.tile([128, 512], bf, tag="q0r")
                        nc.scalar.copy(q0r[:], q0p[:])
                        q1r = rawp.tile([128, 512], bf, tag="q1r")
                        nc.scalar.copy(q1r[:], q1p[:])
                        kvr = rawp.tile([128, 512], bf, tag="kvr")
                        nc.scalar.copy(kvr[:], kvp[:])
                        if tail_args is not None:
                            ph1_tail(*tail_args)
                        tail_args = (b, st, s0, q0r, q1r, kvr)
                ph1_tail(*tail_args)

            # ---------------- phase 2: attention + wo ----------------
            # PSUM budget (8 banks): scores pair-tiles [128,2,512] x2 bufs
            # (4 banks) + 2 OT accumulators (2 banks) + wo psum x2 (2 banks).
            with (
                tc.tile_pool(name="pp", bufs=3) as ppool,
                tc.tile_pool(name="pkd", bufs=2) as pkd,
                tc.tile_pool(name="rcp", bufs=2) as rcp,
                tc.tile_pool(name="scr", bufs=2) as scrp,
                tc.tile_pool(name="wsb", bufs=4) as wsp,
                tc.tile_pool(name="ps_s", bufs=2, space="PSUM") as pss,
                tc.tile_pool(name="ps_o0", bufs=1, space="PSUM") as po0,
                tc.tile_pool(name="ps_o1", bufs=1, space="PSUM") as po1,
                tc.tile_pool(name="ps_w", bufs=2, space="PSUM") as pwo,
            ):
                opool = (po0, po1)

                wo_units = []  # deferred wo sub-stages, drained inside kt loops

                def flush_wo(k):
                    for _ in range(min(k, len(wo_units))):
                        wo_units.pop(0)()

                def attn_pass(b, qs, pair):
                    """Scores+exp+AV for heads (2*pair, 2*pair+1); returns OT tiles."""
                    q0 = qs * 512
                    kts = sched[qs]
                    # Masked blocks (whose exp->mask->AV chain includes GPSIMD)
                    # run mid-loop where the pipeline hides them; the final AV
                    # then has the shortest possible dependency chain.
                    kts = ([kts[0]] + [e for e in kts[1:] if e[1] is not None]
                           + [e for e in kts[1:] if e[1] is None])
                    first_kt, last_kt = kts[0][0], kts[-1][0]
                    ots = [opool[i].tile([HD + 1, 512], f32, name=f"ot{i}")
                           for i in range(2)]

                    def emit_av(pt, c0, kt):
                        for i in range(2):
                            nc.tensor.matmul(
                                ots[i][:, c0:512],
                                lhsT=vone_sb[:, b, kt, :],
                                rhs=pt[:, i, c0:512],
                                start=(kt == first_kt), stop=(kt == last_kt))

                    # AV is software-pipelined one kt behind the scores so the
                    # in-order PE never sits through the exp(+mask) chain.
                    pending_av = None
                    for kt, mi, jlo in kts:
                        c0 = jlo * 128
                        sp = pss.tile([128, 2, 512], f32, tag="sp")
                        for i in range(2):
                            r0 = i * 64  # row strip: the two heads' scores
                            nc.tensor.matmul(  # run concurrently on the PE
                                sp[:, i, c0:512],
                                lhsT=kt_sb[r0:r0 + 64, b, kt * 128:(kt + 1) * 128],
                                rhs=qt_sb[r0:r0 + 64, b, pair, q0 + c0:q0 + 512],
                                start=True, stop=True)
                        if pending_av is not None:
                            emit_av(*pending_av)
                        # interleave prior-stripe wo work to keep the PE dense
                        # through the ACT-gated exp pipeline (HAM stays warm),
                        # but hold some units back for the pass-end chains
                        if len(wo_units) > 10:
                            flush_wo(2)
                        elif len(wo_units) > 6:
                            flush_wo(1)
                        pt = ppool.tile([128, 2, 512], bf, tag="pt")
                        nc.scalar.activation(pt[:, :, c0:512], sp[:, :, c0:512],
                                             AF.Exp, scale=1.0 / np.sqrt(HD))
                        if mi is not None:
                            # GPSIMD is otherwise idle; keep the mask mults off DVE
                            for i in range(2):
                                nc.gpsimd.tensor_mul(pt[:, i, c0:512], pt[:, i, c0:512],
                                                     mt_sb[:, mi, c0:512])
                        pending_av = (pt, c0, kt)
                    # PE filler while the last exp completes
                    flush_wo(4)
                    emit_av(*pending_av)
                    return ots

                def norm_pack(pair, ots, packed):
                    """1/rowsum -> broadcast -> packed[dh-in-pair, pair, q]."""
                    flush_wo(3)  # PE filler while DVE runs the recip chain
                    bc = pss.tile([128, 2, 512], f32, tag="sp")  # borrow a slot
                    rc = rcp.tile([128, 2048], f32, tag="rc")
                    for i in range(2):
                        h = 2 * pair + i
                        pb, po = h // 2, (h % 2) * 64
                        # ~18-bit reciprocal, 5x faster than exact; denominators
                        # are finite and >= exp(max score) so edge cases can't hit.
                        # The staging copy also shifts the denominator row from
                        # partition 64 down to partition 0 so the custom DVE op
                        # and the K=1 broadcast matmul run on base-0 paths.
                        nc.vector.tensor_copy(rc[0:1, i * 512:(i + 1) * 512],
                                              ots[i][64:65, :])
                        nc.vector.reciprocal_approx_fast(
                            rc[0:1, 1024 + i * 512:1024 + (i + 1) * 512],
                            rc[0:1, i * 512:(i + 1) * 512])
                        # rank-1 broadcast of the recip row to all partitions
                        nc.tensor.matmul(bc[:, i, :], lhsT=ones_sb[0:1, :],
                                         rhs=rc[0:1, 1024 + i * 512:1024 + (i + 1) * 512],
                                         start=True, stop=True)
                        # DVE can read only one PSUM operand per op: stage OT in
                        # SBUF (the copy also partition-shifts the upper-half
                        # head into place), then multiply by the PSUM broadcast.
                        sc = scrp.tile([128, 512], bf, tag="sc")
                        nc.vector.tensor_copy(sc[po:po + 64, :], ots[i][0:64, :])
                        nc.vector.tensor_mul(packed[po:po + 64, pb, :],
                                             sc[po:po + 64, :], bc[po:po + 64, i, :])

                def make_wo_unit(b, qs, packed, j, nb):
                    def unit():
                        wp = pwo.tile([128, 512], f32)
                        nc.tensor.matmul(wp[:], lhsT=packed[:, 0, j * 128:(j + 1) * 128],
                                         rhs=wo_sb[:, 0, nb * 512:(nb + 1) * 512],
                                         start=True, stop=False)
                        nc.tensor.matmul(wp[:], lhsT=packed[:, 1, j * 128:(j + 1) * 128],
                                         rhs=wo_sb[:, 1, nb * 512:(nb + 1) * 512],
                                         start=False, stop=True)
                        wsb = wsp.tile([128, 512], f16)
                        nc.vector.tensor_copy(wsb[:], wp[:])
                        nc.sync.dma_start(out.ap()[b, qs, j, nb], wsb[:])
                    return unit

                for b in range(B):
                    for qs in range(QS_TILES):
                        packed = pkd.tile([128, 2, 512], bf, tag="packed")
                        ots = attn_pass(b, qs, 0)
                        norm_pack(0, ots, packed)
                        ots = attn_pass(b, qs, 1)
                        norm_pack(1, ots, packed)
                        wo_units.extend(make_wo_unit(b, qs, packed, j, nb)
                                        for j in range(4) for nb in range(4))
                while wo_units:
                    flush_wo(4)
    nc.compile()
    return nc


def kernel(x, wq, wk, wv, wo, freqs, mask, start_pos):
    sys.path.insert(0, "/opt/trn_rl_repo")
    from concourse.bass_utils import run_bass_kernel_spmd

    x = np.asarray(x, dtype=np.float32)
    per_core, sched, U = _host_prepare(
        x, np.asarray(wq, np.float32), np.asarray(wk, np.float32),
        np.asarray(wv, np.float32), np.asarray(wo, np.float32),
        np.asarray(freqs, np.float32), np.asarray(mask, np.float32))

    nc = _build_program(sched, U)

    trace = bool(int(os.environ.get("BASSKERNEL_TRACE", "0")))
    if trace and "antenv.axon_hooks" not in sys.modules:
        # profile-hook shim (the trimmed antenv package lacks axon_hooks)
        try:
            import types

            if "/root/.axon_site" not in sys.path:
                sys.path.insert(0, "/root/.axon_site")
            from trn_agent_boot.trn_boot import _ntff_profile_via_ctypes

            _hook = _ntff_profile_via_ctypes("/opt/axon/libaxon_pjrt.so")
            _mod = types.ModuleType("antenv.axon_hooks")
            _mod.get_axon_ntff_profile_hook = lambda: _hook
            _mod.set_axon_ntff_profile_hook = lambda h: None
            sys.modules["antenv.axon_hooks"] = _mod
        except Exception:
            trace = False
    res = run_bass_kernel_spmd(nc, per_core, core_ids=list(range(NCORES)),
                               trace=trace)
    if trace:
        kernel._last_exec_time_ns = res.exec_time_ns
        kernel._last_profile = res.profile_json
    acc = res.results[0]["out"].astype(np.float64)
    for c in range(1, NCORES):
        acc += res.results[c]["out"].astype(np.float64)
    return _untile_out(acc).astype(np.float32)
